# revision 1
# baseline (speedup 1.0000x reference)
"""DTM (distance-to-measure) layer kernel for Trainium2, 8 NeuronCores.

For each of 25600 grid points: squared distances to 4096 points, take the
41 smallest, dtm = sqrt((sum40 + 0.96*d2_41) / 40.96).

Distance matrix via one augmented matmul per tile on the tensor engine:
  -d2[m,n] = 2*gx[m]*xx[n] + 2*gy[m]*xy[n] - |x[n]|^2 - |g[m]|^2
Each fp32 factor is split into 3 bf16 terms (hi/mid/lo); the 6 significant
cross products are stacked along the contraction axis (K=24), giving fp32-
level accuracy at bf16 matmul speed (validated: max |diff| vs fp32 ~9e-8).

Top-41 selection per row: 32 segmented max8 ops produce 256 candidates
(top-8 of each 128-column segment; exact containment of the global top-41
unless one segment holds >=9 of them), then 5 rounds of max8+match_replace
plus a final max8 select the 41 smallest. Grid axis sharded 3200 rows/core.
"""

import numpy as np
import ml_dtypes

import concourse.bass as bass
import concourse.bacc as bacc
import concourse.tile as tile
from concourse import mybir
from concourse.bass_utils import run_bass_kernel_spmd

F32 = mybir.dt.float32
BF16 = mybir.dt.bfloat16

N_CORES = 8
H, W = 160, 160
HW = H * W            # 25600 grid points
N = 4096              # point cloud size
S = HW // N_CORES     # 3200 grid rows per core
P = 128               # partitions per tile
NT = S // P           # 25 tiles per core
KC = 24               # contraction: 6 bf16 cross-product terms x 4 rows
NSEG = 16             # segments per row for candidate generation
SEG = N // NSEG       # 128
NCAND = NSEG * 8      # 256 candidates
BOUND = 0.01 * N      # 40.96
NEG_INF = -1e30


def _build_program():
    nc = bacc.Bacc("TRN2", target_bir_lowering=False, debug=False)
    # lhsT' (24, S) and rhs' (24, N) packed side by side -> one DMA
    params = nc.declare_dram_parameter("params", [KC, S + N], BF16, isOutput=False)
    out = nc.declare_dram_parameter("out", [S], F32, isOutput=True)

    with tile.TileContext(nc) as tc:
        with (
            tc.tile_pool(name="const", bufs=1) as const_pool,
            tc.tile_pool(name="psum", bufs=2, space="PSUM") as psum_pool,
            tc.tile_pool(name="d2", bufs=4) as d2_pool,
            tc.tile_pool(name="cand", bufs=4) as cand_pool,
            tc.tile_pool(name="mr", bufs=3) as mr_pool,
            tc.tile_pool(name="small", bufs=12) as small_pool,
        ):
            par_sb = const_pool.tile([KC, S + N], BF16)
            nc.sync.dma_start(par_sb[:], params[:])
            lh_sb = par_sb[:, 0:S]
            rhs_sb = par_sb[:, S:S + N]

            out_v = out[:].rearrange("(t p) -> t p", p=P)  # (NT, 128)
            pending = None

            for t in range(NT):
                cands = cand_pool.tile([P, NCAND], F32)
                halves = []
                for h in range(2):
                    ps = psum_pool.tile([P, N // 2], F32)
                    for j in range(4):
                        nc.tensor.matmul(
                            ps[:, j * 512:(j + 1) * 512],
                            lh_sb[:, t * P:(t + 1) * P],
                            rhs_sb[:, h * 2048 + j * 512:h * 2048 + (j + 1) * 512],
                        )
                    d2h = d2_pool.tile([P, N // 2], F32, tag=f"d2h{h}")
                    nc.scalar.copy(d2h[:], ps[:])
                    halves.append(d2h)

                for h in range(2):
                    for s in range(NSEG // 2):
                        g = h * (NSEG // 2) + s
                        nc.vector.max(
                            cands[:, 8 * g:8 * g + 8],
                            halves[h][:, SEG * s:SEG * (s + 1)],
                        )

                mr = mr_pool.tile([P, 48], F32)
                for r in range(5):
                    nc.vector.max(mr[:, 8 * r:8 * r + 8], cands[:])
                    nc.vector.match_replace(
                        cands[:], mr[:, 8 * r:8 * r + 8], cands[:], NEG_INF
                    )
                nc.vector.max(mr[:, 40:48], cands[:])
                tau_pos = small_pool.tile([P, 1], F32)
                nc.vector.tensor_scalar_mul(tau_pos[:], mr[:, 40:41], -1.0)

                def epilogue(halves=halves, tau_pos=tau_pos, t=t):
                    # relu identity: dtm^2 = tau - sum relu(tau - d2)/BOUND
                    a1a = small_pool.tile([P, 1], F32)
                    a1b = small_pool.tile([P, 1], F32)
                    nc.scalar.activation(
                        halves[0][:], halves[0][:], mybir.ActivationFunctionType.Relu,
                        bias=tau_pos[:], scale=1.0, accum_out=a1a[:],
                    )
                    nc.scalar.activation(
                        halves[1][:], halves[1][:], mybir.ActivationFunctionType.Relu,
                        bias=tau_pos[:], scale=1.0, accum_out=a1b[:],
                    )
                    a1 = small_pool.tile([P, 1], F32)
                    nc.vector.tensor_add(a1[:], a1a[:], a1b[:])
                    comb = small_pool.tile([P, 1], F32)
                    nc.vector.scalar_tensor_tensor(
                        comb[:], tau_pos[:], BOUND, a1[:],
                        op0=mybir.AluOpType.mult, op1=mybir.AluOpType.subtract,
                    )
                    dtm = small_pool.tile([P, 1], F32)
                    nc.scalar.activation(
                        dtm[:], comb[:], mybir.ActivationFunctionType.Sqrt,
                        scale=1.0 / BOUND,
                    )
                    nc.sync.dma_start(out_v[t], dtm[:, 0])

                if pending is not None:
                    pending()
                pending = epilogue
            pending()

    if not nc.is_finalized():
        nc.finalize()
    return nc


def _make_grid():
    # mirrors reference make_grid: x ascending, y descending, meshgrid 'xy'
    x_seq = np.linspace(-0.1, 0.1, W, dtype=np.float32)
    y_seq = np.linspace(0.1, -0.1, H, dtype=np.float32)
    xc, yc = np.meshgrid(x_seq, y_seq, indexing="xy")
    return np.concatenate(
        [xc.reshape(-1, 1), yc.reshape(-1, 1)], axis=1
    ).astype(np.float32)


def _split3(v):
    bf = ml_dtypes.bfloat16
    h = v.astype(bf).astype(np.float32)
    m = (v - h).astype(bf).astype(np.float32)
    l = (v - h - m).astype(bf).astype(np.float32)
    return h, m, l


def _prep_inputs(x, grid):
    x = np.asarray(x, dtype=np.float32)
    grid = np.asarray(grid, dtype=np.float32)
    gx, gy = grid[:, 0], grid[:, 1]
    g2 = gx * gx + gy * gy
    A = np.stack(
        [2.0 * gx, 2.0 * gy, -np.ones(HW, np.float32), -g2]
    ).astype(np.float32)  # (4, HW)
    xx, xy = x[:, 0], x[:, 1]
    x2 = xx * xx + xy * xy
    B = np.stack([xx, xy, x2, np.ones(N, np.float32)]).astype(np.float32)  # (4, N)
    # 3-way bf16 split; 6 significant cross products along K
    Ah, Am, Al = _split3(A)
    Bh, Bm, Bl = _split3(B)
    A24 = np.concatenate([Ah, Ah, Am, Ah, Am, Al]).astype(ml_dtypes.bfloat16)
    B24 = np.concatenate([Bh, Bm, Bh, Bl, Bm, Bh]).astype(ml_dtypes.bfloat16)
    return [
        {
            "params": np.ascontiguousarray(
                np.concatenate([A24[:, c * S:(c + 1) * S], B24], axis=1)
            )
        }
        for c in range(N_CORES)
    ]


def _install_profile_hook():
    """Shim antenv.axon_hooks (absent in this image) so trace=True works."""
    import sys as _sys
    import types as _types
    try:
        import antenv
        try:
            from antenv.axon_hooks import get_axon_ntff_profile_hook  # noqa: F401
            return
        except ImportError:
            pass
        hooks = _types.ModuleType("antenv.axon_hooks")
        _state = {"hook": None}
        hooks.set_axon_ntff_profile_hook = lambda h: _state.__setitem__("hook", h)
        hooks.get_axon_ntff_profile_hook = lambda: _state["hook"]
        _sys.modules["antenv.axon_hooks"] = hooks
        antenv.axon_hooks = hooks
        from trn_agent_boot.trn_boot import _ntff_profile_via_ctypes
        hook = _ntff_profile_via_ctypes("/opt/axon/libaxon_pjrt.so")
        if hook is not None:
            hooks.set_axon_ntff_profile_hook(hook)
    except Exception as e:  # profiling is best-effort
        print("profile hook install failed:", e)


def run(x, grid=None, trace=False):
    """Returns (dtm (160,160) float32, exec_time_ns or None)."""
    if trace:
        _install_profile_hook()
    if grid is None:
        grid = _make_grid()
    in_maps = _prep_inputs(x, grid)
    nc = _build_program()
    res = run_bass_kernel_spmd(nc, in_maps, list(range(N_CORES)), trace=trace)
    dtm = np.concatenate([res.results[c]["out"] for c in range(N_CORES)])
    return dtm.reshape(H, W).astype(np.float32), res.exec_time_ns


def kernel(x, grid=None):
    out, _ = run(x, grid)
    return out

